# revision 16
# baseline (speedup 1.0000x reference)
"""Trainium2 Bass kernel for nn_MLPMHA (sparse_attention / squared-ReLU MLP-MHA).

Reference computation (B=4, T=2048, C=1024, QH=4, D=256, S=4C=4096):
    x   = layernorm(residual) * g + b
    q_h = x[:, h*D:(h+1)*D]                     per head h
    k   = w_fc.reshape(S, D)                    keys   (shared across heads)
    v   = w_proj.T.reshape(S, D)                values (shared across heads)
    out = residual + concat_h( relu(q_h @ k.T)^2 @ v )

Equivalent blocked form used here (cc = 0..3 indexes 256-wide column chunks
of w_fc / row chunks of w_proj; all matmuls are plain GEMMs):
    A_{h,cc}  = x_h @ w_fc[:, cc*D:(cc+1)*D].T          (T, C)
    out_h     = sum_cc relu(A_{h,cc})^2 @ w_proj[cc*D:(cc+1)*D, :].T   (T, D)

Sharding: pure data parallel over the 8192 = B*T token rows; each of the 8
cores processes 1024 rows with full (transposed) weights resident in SBUF.

On-core dataflow (PSUM accumulation fp32; matmul operands bf16 by default —
1 cycle/row on the PE like f32r, but half the SBUF/DMA traffic, FWL weight
loads, 2x DVE throughput, and 1-cycle PE transposes; measured accuracy cost
~3e-3 relative vs the 2e-2 gate):
    phase A: DMA residual rows into a persistent buffer, LayerNorm
             (bn_stats, in two tile-groups), apply ln_g via one broadcast
             multiply (ln_b is folded into a per-score bias = ln_b @ w_fc,
             host-precomputed), then one 3D DMA-xbar transpose per token
             tile moves xn into xT[c, t] layout (a_dmat; no PE cycles).
    phase B: per (h, cc, i-chunk): A^T tile = wfcT_chunk.T @ xT  (PSUM),
             relu(.+bias)^2 via ACT-relu + DVE-square (recipe mix),
             out^T PSUM accumulation over all (cc, i).
    phase C: drain out^T PSUM via ACT/DVE copies (oc_split), one 3D
             DMA-xbar transpose per (dd, tch) back to [t, c] (epi_dmat2),
             DVE-add into the residual buffer, DMA out per 4-tile group.
             The PE stream is pure matmul+ldweights (1024 + 1024 instrs);
             measured ~27 ns/PE-instr of fixed overhead puts the stream
             floor at ~273 us (mm_only variant) vs the 218 us row count.

For the benchmark reps-loop, consecutive reps are software-pipelined across
double-buffered residual/xT sets: rep i+1's phase A is emitted as closures
that ride inside rep i's phase-B instruction streams, so its DMA/LN work
hides behind rep i's matmuls ("xrep").
"""

import numpy as np

import concourse.bass as bass
import concourse.tile as tile
from concourse import mybir, bacc
from concourse.bass_utils import run_bass_kernel_spmd
from concourse.masks import make_identity

P = 128
C = 1024
D = 256
QH = 4
NCC = 4          # column chunks of w_fc (S = NCC * C kv entries)
N_CORES = 8
ROWS = 1024      # token rows per core (8192 / 8)
NT = ROWS // P   # 8 row tiles per core
EPS = 1e-5

F32 = mybir.dt.float32
F32R = mybir.dt.float32r
BF16 = mybir.dt.bfloat16

_NC_CACHE = {}

# tuning knobs (A/B tested on hardware)
CONFIG = {
    "lookahead": 2,        # software-pipeline depth for mm2 behind mm1
    "lookahead0": 2,       # mm2 lag ramp target at each head start (>=
                           # lookahead; gives the previous head's PSUM
                           # drain more slack before mm2 start=True)
    "pools": (2, 4, 0),    # psA, psO, psT bufs (psT=0 => share psA slots)
                           # wide_mm: po tiles are 2 banks, so psO bufs=2
                           # (use (2,4,0) when wide_mm=False)
    "pt_bufs": 4,          # ptpool bufs
    "work_bufs": 3,        # work pool bufs
    "defer_epi": True,     # emit head epilogue after next head's first mm1s
    "epi_at": 3,           # block index in next head where epilogue lands
    "bf16": True,          # bf16 matmul operands (weights, xT, pT) + transposes
    "mix": "XXXXZ",        # relu^2 recipe cycle: X = ACT-relu + DVE-square,
                           # Z = DVE-relu + DVE-square, Y = DVE-relu + ACT-sq
    "xn_dve": True,        # layernorm-apply on DVE instead of ACT
    "aff_split": True,     # alternate phase-A copy-backs between ACT and DVE
    "interleave_a": True,  # ride phase-A och4-7 transposes in the PE stream
    "xrep": True,          # software-pipeline reps across double buffers
    "xa_stride": 8,        # blocks between cross-rep phase-A closures
    "epi_dmat": False,     # epilogue transposes via DMA xbar instead of PE
    "w_reorder": True,     # mm1 shares each stationary across token halves
    "wide_relu": True,     # one 1024-wide relu^2 per block (2-bank psA tiles)
    "wide_mm": False,      # 1024-wide matmuls: REJECTED by walrus ISA check
                           # (moving >512 => out crosses a PSUM bank); keep off
    "a_dmat": True,        # phase-A transposes via one 3D DMA-xbar transpose
                           # per token tile (frees PE + the ACT/DVE copy-backs)
    "epi_dmat2": True,     # epilogue transposes via one 3D DMA-xbar transpose
                           # per (dd, tch) on the ACT HWDGE queue
    "oc_split": True,      # epilogue PSUM-drain copies alternate ACT/DVE so
                           # the next head's first mm2 unblocks sooner
}


def _phase_a_closures(env, buf, defer_grp1):
    """Phase A for buffer set `buf` as an ordered list of closures.

    Returns (closures, a_jobs): running every closure in order (at any
    spacing) then every job emits the full LayerNorm + transpose of x into
    buf['xT'].  a_jobs (och 4-7 transpose groups) may be deferred into the
    same rep's phase-B stream when `defer_grp1`.
    """
    nc, work, cfg = env["nc"], env["work"], CONFIG
    TDT = BF16 if cfg["bf16"] else F32
    resid_sb, par = buf["resid"], buf["par"]
    resid, gbc_sb, eps_t = env["resid"], env["gbc_sb"], env["eps_t"]

    stats_all = work.tile([P, NT, 2, 6], F32, name=f"sta{par}",
                          tag=f"stats{par}", bufs=1)
    mv_all = work.tile([P, NT, 2], F32, name=f"mva{par}", tag=f"mv{par}",
                       bufs=1)
    nmr_all = work.tile([P, NT], F32, name=f"nmra{par}", tag=f"nmr{par}",
                        bufs=1)
    xn_tiles = {}
    cls = []

    def _dmas():
        for tt in range(NT):
            nc.sync.dma_start(resid_sb[:, tt, :],
                              resid[tt * P:(tt + 1) * P, :])
    cls.append(_dmas)

    def _bn(tg):
        for tt in range(tg * 4, (tg + 1) * 4):
            nc.vector.bn_stats(stats_all[:, tt, 0, :], resid_sb[:, tt, 0:512])
            nc.vector.bn_stats(stats_all[:, tt, 1, :],
                               resid_sb[:, tt, 512:1024])

    def _fin(tg):
        tsl4 = slice(tg * 4, (tg + 1) * 4)
        for tt in range(tg * 4, (tg + 1) * 4):
            nc.vector.bn_aggr(mv_all[:, tt, :], stats_all[:, tt, :, :])
        # rstd = 1/sqrt(var+eps), nmr = -mu*rstd (xn = r*rstd + nmr)
        nc.scalar.activation(mv_all[:, tsl4, 1], mv_all[:, tsl4, 1],
                             mybir.ActivationFunctionType.Sqrt,
                             bias=eps_t[:], scale=1.0)
        nc.vector.reciprocal(mv_all[:, tsl4, 1], mv_all[:, tsl4, 1])
        nc.vector.tensor_tensor(out=nmr_all[:, tsl4], in0=mv_all[:, tsl4, 0],
                                in1=mv_all[:, tsl4, 1],
                                op=mybir.AluOpType.mult)
        nc.vector.tensor_scalar_mul(out=nmr_all[:, tsl4],
                                    in0=nmr_all[:, tsl4], scalar1=-1.0)

    def _xn(tt):
        xn = work.tile([P, C], TDT, name=f"xn{par}_{tt}", tag=f"xn{par}",
                       bufs=NT)
        xn_tiles[tt] = xn
        if cfg["xn_dve"]:
            nc.vector.tensor_scalar(out=xn[:], in0=resid_sb[:, tt, :],
                                    scalar1=mv_all[:, tt, 1:2],
                                    scalar2=nmr_all[:, tt:tt + 1],
                                    op0=mybir.AluOpType.mult,
                                    op1=mybir.AluOpType.add)
        else:
            nc.scalar.activation(xn[:], resid_sb[:, tt, :],
                                 mybir.ActivationFunctionType.Identity,
                                 bias=nmr_all[:, tt:tt + 1],
                                 scale=mv_all[:, tt, 1:2])
        # x_hat * g: ln_g varies along the free (channel) axis here, so one
        # broadcast-multiply handles all 8 chunks (ln_b is folded into the
        # score bias bias_sb = ln_b @ w_fc, applied at the relu)
        nc.vector.tensor_mul(out=xn[:], in0=xn[:], in1=gbc_sb[:])

    def _tgroup(tt, grp):
        if cfg["a_dmat"]:
            # one DMA-xbar transpose moves the whole [128-token, 1024-ch]
            # xn tile into xT layout: out[p, och, j] = xn[j, och*128 + p]
            # (grp carries the full tile when 0; grp 1 is a no-op)
            if grp == 0:
                nc.sync.dma_start_transpose(
                    buf["xT"][:, 0:8, tt * P:(tt + 1) * P], xn_tiles[tt][:])
            return
        # transposes grouped 4-per-PSUM-bank with one grouped copy, to cut
        # the non-PE op count (errata makes small ops expensive)
        psT, _ptag, ident = env["psT"], env["_ptag"], env["ident"]
        pst = psT.tile([P, 512], TDT, name=f"psx{par}_{tt}_{grp}",
                       tag=_ptag)
        xn = xn_tiles[tt]
        for k in range(4):
            och = grp * 4 + k
            nc.tensor.transpose(pst[:, k * P:(k + 1) * P],
                                xn[:, och * P:(och + 1) * P], ident[:])
        dst = buf["xT"][:, grp * 4:(grp + 1) * 4, tt * P:(tt + 1) * P]
        if cfg["aff_split"] and (tt + grp) % 2 == 1:
            nc.vector.tensor_copy(dst, pst.rearrange("p (k c) -> p k c", k=4))
        else:
            nc.scalar.activation(dst, pst.rearrange("p (k c) -> p k c", k=4),
                                 mybir.ActivationFunctionType.Identity)

    # stats + xn in two tile-groups so tiles 0-3 are not gated on the last
    # residual DMA; och-0-3 transposes come first (heads 0/1's channels)
    for tg in range(2):
        cls.append(lambda tg=tg: _bn(tg))
        cls.append(lambda tg=tg: _fin(tg))
        for tt in range(tg * 4, (tg + 1) * 4):
            cls.append(lambda tt=tt: _xn(tt))
    for tt in range(NT):
        cls.append(lambda tt=tt: _tgroup(tt, 0))
    a_jobs = [(tt, 1) for tt in range(NT)]
    if not defer_grp1 or cfg["a_dmat"]:
        if not cfg["a_dmat"]:
            for tt in range(NT):
                cls.append(lambda tt=tt: _tgroup(tt, 1))
        a_jobs = []
    return cls, a_jobs, _tgroup


def _phase_bc(env, buf, rep, variant='full', a_jobs=(), tgroup_fn=None,
              nbuf_closures=()):
    """Phase B (matmuls + relu^2 + out^T accumulation) + phase C for buffer
    set `buf`.  Optionally interleaves same-rep transpose jobs (`a_jobs`)
    and the next rep's phase-A closures (`nbuf_closures`) into the block
    stream."""
    nc, work, ptpool = env["nc"], env["work"], env["ptpool"]
    psA, psO, psT = env["psA"], env["psO"], env["psT"]
    wfcT_sb, wprojT_sb = env["wfcT_sb"], env["wprojT_sb"]
    bias_sb, ident, out = env["bias_sb"], env["ident"], env["out"]
    _ptag, pT_dummy = env["_ptag"], env["pT_dummy"]
    cfg = CONFIG
    MDT = BF16 if cfg["bf16"] else F32R
    TDT = BF16 if cfg["bf16"] else F32
    xT_sb, resid_sb, par = buf["xT"], buf["resid"], buf["par"]
    a_jobs = list(a_jobs)
    nbuf_closures = list(nbuf_closures)

    pending_epi = [None]

    def _flush_epi():
        if pending_epi[0] is not None:
            pending_epi[0]()
            pending_epi[0] = None

    tile_ctr = [0]
    blk_ctr = [0]

    for h in range(0 if variant == 'a_only' else QH):
        if cfg["wide_mm"]:
            # po[dd] is one 2-bank PSUM tile covering all 1024 tokens
            po = [psO.tile([P, 1024], F32, name=f"po{par}_{rep}_{h}_{dd}",
                           tag="po") for dd in range(2)]
        else:
            po = [[psO.tile([P, 512], F32,
                            name=f"po{par}_{rep}_{h}_{dd}_{tch}",
                            tag="po") for tch in range(2)] for dd in range(2)]

        def _mm1_raw(cc, ich):
            """mm1 without the nonlinearity (diagnostic variants)."""
            isl = slice(ich * P, (ich + 1) * P)
            if cfg["wide_mm"]:
                ps = psA.tile([P, 1024], F32,
                              name=f"psaq{par}_{rep}_{h}_{cc}_{ich}",
                              tag="psa")
                for wi in range(2):
                    nc.tensor.matmul(ps[:], wfcT_sb[:, cc * 2 + wi, isl],
                                     xT_sb[:, h * 2 + wi, :],
                                     start=(wi == 0), stop=(wi == 1))
                return
            for tch in range(2):
                ps = psA.tile([P, 512], F32,
                              name=f"psaq{par}_{rep}_{h}_{cc}_{ich}_{tch}",
                              tag="psa")
                tsl = slice(tch * 512, (tch + 1) * 512)
                nc.tensor.matmul(ps[:], wfcT_sb[:, cc * 2 + 0, isl],
                                 xT_sb[:, h * 2 + 0, tsl],
                                 start=True, stop=False)
                nc.tensor.matmul(ps[:], wfcT_sb[:, cc * 2 + 1, isl],
                                 xT_sb[:, h * 2 + 1, tsl],
                                 start=False, stop=True)

        def _mm1(cc, ich):
            pT = ptpool.tile([P, ROWS], MDT,
                             name=f"pT{par}_{rep}_{h}_{cc}_{ich}", tag="pT")
            isl = slice(ich * P, (ich + 1) * P)
            bias_ap = bias_sb[:, h, cc, ich:ich + 1]
            if cfg["wide_relu"]:
                # one 2-bank PSUM tile for both token halves -> a single
                # 1024-wide relu + square per block (halves ACT/DVE op
                # count and engine energy; each matmul still writes within
                # one bank)
                ps = psA.tile([P, 1024], F32,
                              name=f"psw{par}_{rep}_{h}_{cc}_{ich}",
                              tag="psa")
                if cfg["wide_mm"]:
                    # one 1024-wide matmul per contraction chunk
                    for wi in range(2):
                        nc.tensor.matmul(ps[:], wfcT_sb[:, cc * 2 + wi, isl],
                                         xT_sb[:, h * 2 + wi, :],
                                         start=(wi == 0), stop=(wi == 1))
                else:
                    for wi in range(2):
                        for tch in range(2):
                            tsl = slice(tch * 512, (tch + 1) * 512)
                            nc.tensor.matmul(ps[:, tsl],
                                             wfcT_sb[:, cc * 2 + wi, isl],
                                             xT_sb[:, h * 2 + wi, tsl],
                                             start=(wi == 0), stop=(wi == 1))
                rl = work.tile([P, 1024], MDT,
                               name=f"rlw{par}_{rep}_{h}_{cc}_{ich}",
                               tag="rl")
                nc.scalar.activation(rl[:], ps[:],
                                     mybir.ActivationFunctionType.Relu,
                                     bias=bias_ap, scale=1.0)
                nc.vector.tensor_mul(out=pT[:], in0=rl[:], in1=rl[:])
                tile_ctr[0] += 2
                return pT
            pss = []
            if cfg["w_reorder"]:
                # both token halves against one stationary before switching:
                # halves the LDWEIGHTS rate (interleaved accumulation groups
                # in two PSUM banks)
                for tch in range(2):
                    pss.append(psA.tile(
                        [P, 512], F32,
                        name=f"psa{par}_{rep}_{h}_{cc}_{ich}_{tch}",
                        tag="psa"))
                for wi in range(2):
                    for tch in range(2):
                        tsl = slice(tch * 512, (tch + 1) * 512)
                        nc.tensor.matmul(pss[tch][:],
                                         wfcT_sb[:, cc * 2 + wi, isl],
                                         xT_sb[:, h * 2 + wi, tsl],
                                         start=(wi == 0), stop=(wi == 1))
            for tch in range(2):
                if cfg["w_reorder"]:
                    ps = pss[tch]
                else:
                    ps = psA.tile(
                        [P, 512], F32,
                        name=f"psa{par}_{rep}_{h}_{cc}_{ich}_{tch}",
                        tag="psa")
                    tsl = slice(tch * 512, (tch + 1) * 512)
                    nc.tensor.matmul(ps[:], wfcT_sb[:, cc * 2 + 0, isl],
                                     xT_sb[:, h * 2 + 0, tsl],
                                     start=True, stop=False)
                    nc.tensor.matmul(ps[:], wfcT_sb[:, cc * 2 + 1, isl],
                                     xT_sb[:, h * 2 + 1, tsl],
                                     start=False, stop=True)
                tsl = slice(tch * 512, (tch + 1) * 512)
                # pT = relu(ps + bias)^2; recipe per tile from cfg["mix"]
                # (bias = ln_b @ w_fc chunk, the folded layernorm bias)
                tile_ctr[0] += 1
                recipe = cfg["mix"][tile_ctr[0] % len(cfg["mix"])]
                rl = work.tile([P, 512], MDT,
                               name=f"rl{par}_{rep}_{h}_{cc}_{ich}_{tch}",
                               tag="rl")
                if recipe == "X":
                    nc.scalar.activation(rl[:], ps[:],
                                         mybir.ActivationFunctionType.Relu,
                                         bias=bias_ap, scale=1.0)
                    nc.vector.tensor_mul(out=pT[:, tsl], in0=rl[:], in1=rl[:])
                elif recipe == "Z":
                    nc.vector.tensor_scalar(out=rl[:], in0=ps[:],
                                            scalar1=bias_ap, scalar2=0.0,
                                            op0=mybir.AluOpType.add,
                                            op1=mybir.AluOpType.max)
                    nc.vector.tensor_mul(out=pT[:, tsl], in0=rl[:], in1=rl[:])
                else:  # Y
                    nc.vector.tensor_scalar(out=rl[:], in0=ps[:],
                                            scalar1=bias_ap, scalar2=0.0,
                                            op0=mybir.AluOpType.add,
                                            op1=mybir.AluOpType.max)
                    nc.scalar.activation(pT[:, tsl], rl[:],
                                         mybir.ActivationFunctionType.Square)
            return pT

        def _mm2(cc, ich, pT):
            first = (cc == 0 and ich == 0)
            last = (cc == NCC - 1 and ich == 7)
            for dd in range(2):
                wsl = slice(cc * D + dd * P, cc * D + (dd + 1) * P)
                if cfg["wide_mm"]:
                    nc.tensor.matmul(po[dd][:], wprojT_sb[:, ich, wsl],
                                     pT[:], start=first, stop=last)
                    continue
                # tch inner: consecutive matmuls share the stationary
                # operand, halving the LDWEIGHTS rate
                for tch in range(2):
                    tsl = slice(tch * 512, (tch + 1) * 512)
                    nc.tensor.matmul(po[dd][tch][:],
                                     wprojT_sb[:, ich, wsl], pT[:, tsl],
                                     start=first, stop=last)

        pending = []
        nblk = 0
        for cc in range(NCC):
            for ich in range(8):
                nblk += 1
                blk_ctr[0] += 1
                if nblk == cfg["epi_at"]:
                    # previous head's epilogue lands here, hidden behind
                    # this head's first mm1 blocks
                    _flush_epi()
                if a_jobs and blk_ctr[0] % 4 == 0:
                    # a deferred phase-A transpose group rides along in the
                    # PE stream (och 4-7 are first consumed by head 2)
                    tgroup_fn(*a_jobs.pop(0))
                if nbuf_closures and blk_ctr[0] % cfg["xa_stride"] == 0:
                    # the next rep's phase-A work rides along too (it
                    # touches only the other buffer set)
                    nbuf_closures.pop(0)()
                if variant == 'mm_only':
                    # detached: mm2 reads a pre-set dummy, so PE runs the
                    # pure matmul stream with no DVE/ACT deps
                    _mm2(cc, ich, pT_dummy)
                    _mm1_raw(cc, ich)
                    continue
                pT = _mm1(cc, ich)
                if variant == 'mm1_only':
                    continue
                pending.append((cc, ich, pT))
                la = max(cfg["lookahead"], cfg.get("lookahead0", 0))
                if len(pending) > la:
                    _mm2(*pending.pop(0))
        for args in pending:
            _mm2(*args)

        # epilogue for head h: transpose out^T back, add into resid_sb;
        # after the last head's adds, each 4-token-tile group stores out
        def _epilogue(h=h, po=po):
            if variant not in ('full', 'b_only'):
                return
            for tch in range(2):
                tts = slice(tch * 4, (tch + 1) * 4)
                for dd in range(2):
                    po_src = (po[dd][:, tch * 512:(tch + 1) * 512]
                              if cfg["wide_mm"] else po[dd][tch][:])
                    oc = work.tile([P, 512], TDT,
                                   name=f"oc{par}_{rep}_{h}_{dd}_{tch}",
                                   tag="oc")
                    if cfg["oc_split"] and dd == 1:
                        nc.vector.tensor_copy(oc[:], po_src)
                    else:
                        nc.scalar.activation(
                            oc[:], po_src,
                            mybir.ActivationFunctionType.Identity)
                    csl0 = h * D + dd * P
                    csl = slice(csl0, csl0 + P)
                    if CONFIG["epi_dmat2"]:
                        # one 3D DMA-xbar transpose per (dd, tch):
                        # ocT[p, k, j] = oc[j, k*128 + p]  (contiguous dst)
                        ocT = work.tile([P, 512], TDT,
                                        name=f"ocU{par}_{rep}_{h}_{dd}_{tch}",
                                        tag="ocT")
                        nc.scalar.dma_start_transpose(
                            ocT.rearrange("p (k c) -> p k c", k=4), oc[:])
                        nc.vector.tensor_add(
                            out=resid_sb[:, tts, csl],
                            in0=resid_sb[:, tts, csl],
                            in1=ocT.rearrange("p (k c) -> p k c", k=4))
                    elif CONFIG["epi_dmat"]:
                        # transpose via the DMA xbar (idle ACT queue) to
                        # free PE cycles; SBUF->SBUF, 2-byte dtype only
                        ocT = work.tile([P, 512], TDT,
                                        name=f"ocT{par}_{rep}_{h}_{dd}_{tch}",
                                        tag="ocT")
                        for ts4 in range(4):
                            nc.scalar.dma_start_transpose(
                                ocT[:, ts4 * P:(ts4 + 1) * P],
                                oc[:, ts4 * P:(ts4 + 1) * P])
                        nc.vector.tensor_add(
                            out=resid_sb[:, tts, csl],
                            in0=resid_sb[:, tts, csl],
                            in1=ocT.rearrange("p (k c) -> p k c", k=4))
                    else:
                        pst = psT.tile([P, 512], TDT,
                                       name=f"pso{par}_{rep}_{h}_{dd}_{tch}",
                                       tag=_ptag)
                        for ts4 in range(4):
                            nc.tensor.transpose(
                                pst[:, ts4 * P:(ts4 + 1) * P],
                                oc[:, ts4 * P:(ts4 + 1) * P], ident[:])
                        # one grouped add over the 4 token tiles
                        nc.vector.tensor_add(
                            out=resid_sb[:, tts, csl],
                            in0=resid_sb[:, tts, csl],
                            in1=pst.rearrange("p (k c) -> p k c", k=4))
                if h == QH - 1:
                    for tt in range(tch * 4, (tch + 1) * 4):
                        nc.sync.dma_start(out[tt * P:(tt + 1) * P, :],
                                          resid_sb[:, tt, :])

        if cfg["defer_epi"]:
            _flush_epi()
            pending_epi[0] = _epilogue
        else:
            _epilogue()
    _flush_epi()
    for job in a_jobs:
        tgroup_fn(*job)
    for cl in nbuf_closures:
        cl()


def _phase_abc(env, buf, rep, variant='full'):
    """One full rep: inline phase A (optionally deferring och-4-7 transpose
    groups into this rep's own phase-B stream), then phase B/C."""
    skip_a = variant in ('b_only', 'mm_only', 'mm1_only')
    a_jobs, tgroup_fn = (), None
    if not skip_a:
        defer = CONFIG["interleave_a"] and variant == 'full'
        cls, a_jobs, tgroup_fn = _phase_a_closures(env, buf, defer)
        for cl in cls:
            cl()
    _phase_bc(env, buf, rep, variant, a_jobs, tgroup_fn)


def _build_body(tc, resid, wfcT, wprojT, g_bc, bias_h, out, reps,
                variant='full'):
    nc = tc.nc
    import contextlib
    cfg = CONFIG
    MDT = BF16 if cfg["bf16"] else F32R
    ctx = contextlib.ExitStack()
    with ctx:
        singles = ctx.enter_context(tc.tile_pool(name="singles", bufs=1))
        work = ctx.enter_context(tc.tile_pool(name="work", bufs=cfg["work_bufs"]))
        ptpool = ctx.enter_context(tc.tile_pool(name="ptpool", bufs=cfg["pt_bufs"]))
        psA = ctx.enter_context(tc.tile_pool(name="psA", bufs=cfg["pools"][0], space="PSUM"))
        psO = ctx.enter_context(tc.tile_pool(name="psO", bufs=cfg["pools"][1], space="PSUM"))
        if cfg["pools"][2]:
            psT = ctx.enter_context(tc.tile_pool(name="psT", bufs=cfg["pools"][2], space="PSUM"))
        else:
            psT = psA  # transposes share the psA slots (same tag => same banks)

        # ---- resident tensors -------------------------------------------
        # weights + constants ride the Activation HWDGE queue so the
        # latency-critical residual loads (SP queue) start immediately
        wfcT_sb = singles.tile([P, 8, C], MDT)
        nc.scalar.dma_start(wfcT_sb[:], wfcT.rearrange("(o p) i -> p o i", p=P))
        # ln_g replicated across partitions (host-prepared, matmul dtype)
        gbc_sb = singles.tile([P, C], MDT)
        nc.scalar.dma_start(gbc_sb[:], g_bc)
        # per-(head, cc, i-chunk) score bias = ln_b @ w_fc (host-prepared)
        bias_sb = singles.tile([P, QH, NCC, 8], F32)
        nc.scalar.dma_start(bias_sb[:],
                            bias_h.rearrange("h n (i p) -> p h n i", p=P))
        wprojT_sb = singles.tile([P, 8, C], MDT)
        nc.scalar.dma_start(wprojT_sb[:],
                            wprojT.rearrange("(o p) i -> p o i", p=P))
        ident = singles.tile([P, P], BF16 if cfg["bf16"] else F32)
        make_identity(nc, ident[:])
        eps_t = singles.tile([P, 1], F32)
        nc.vector.memset(eps_t[:], EPS)

        xrep = cfg["xrep"] and variant == 'full' and reps > 1
        bufs = []
        for par in range(2 if xrep else 1):
            bufs.append({
                "par": par,
                "xT": singles.tile([P, 8, ROWS], MDT, name=f"xT{par}"),
                "resid": singles.tile([P, NT, C], F32, name=f"resid{par}"),
            })

        pT_dummy = None
        if variant != 'full':
            # diagnostics-only variants may skip the phases that write these
            pT_dummy = singles.tile([P, ROWS], MDT)
            nc.sync.dma_start(pT_dummy[:], wfcT[0:P, :])
            nc.sync.dma_start(bufs[0]["xT"][:],
                              wfcT.rearrange("(o p) i -> p o i", p=P))
            nc.vector.memset(bufs[0]["resid"][:], 0.0)

        env = {"nc": nc, "work": work, "ptpool": ptpool, "psA": psA,
               "psO": psO, "psT": psT, "resid": resid, "out": out,
               "wfcT_sb": wfcT_sb, "wprojT_sb": wprojT_sb,
               "gbc_sb": gbc_sb, "bias_sb": bias_sb, "ident": ident,
               "eps_t": eps_t, "pT_dummy": pT_dummy,
               "_ptag": "psa" if psT is psA else "pst"}

        # ---- phases A/B/C, repeated `reps` times for benchmarking -------
        # (each rep recomputes from the DMA'd inputs and rewrites the same
        # output, so the result stays correct for any reps >= 1).  reps > 1
        # uses a hardware loop so the instruction count stays constant.
        hint = (mybir.EngineType.PE, mybir.EngineType.Activation,
                mybir.EngineType.DVE, mybir.EngineType.SP,
                mybir.EngineType.Pool)
        nofori = cfg.get("nofori", False)
        if reps == 1:
            _phase_abc(env, bufs[0], 0, variant)
        elif not xrep:
            if nofori:
                for r in range(reps):
                    _phase_abc(env, bufs[0], r, variant)
            else:
                with tc.For_i(0, reps, 1, hint_engines=hint):
                    _phase_abc(env, bufs[0], 0, variant)
        else:
            assert reps % 2 == 0, "xrep software pipeline needs even reps"
            # prologue: phase A for buffer 0 (runs once)
            cls0, _, _ = _phase_a_closures(env, bufs[0], False)
            for cl in cls0:
                cl()

            def _iter(r):
                for par in range(2):
                    cls, jobs, tg = _phase_a_closures(env, bufs[1 - par],
                                                      False)
                    _phase_bc(env, bufs[par], r, variant,
                              a_jobs=jobs, tgroup_fn=tg, nbuf_closures=cls)

            if nofori:
                for r in range(reps // 2):
                    _iter(r)
            else:
                with tc.For_i(0, reps // 2, 1, hint_engines=hint):
                    _iter(0)


def build_nc(reps=1, variant='full'):
    key = (reps, variant, str(sorted(CONFIG.items())))
    if key in _NC_CACHE:
        return _NC_CACHE[key]
    MDT = BF16 if CONFIG["bf16"] else F32R
    nc = bacc.Bacc("TRN2", target_bir_lowering=False, debug=False,
                   num_devices=N_CORES)
    resid = nc.dram_tensor("residual", [ROWS, C], F32, kind="ExternalInput").ap()
    wfcT = nc.dram_tensor("w_fcT", [C, C], MDT, kind="ExternalInput").ap()
    wprojT = nc.dram_tensor("w_projT", [C, C], MDT, kind="ExternalInput").ap()
    g_bc = nc.dram_tensor("g_bc", [P, C], MDT, kind="ExternalInput").ap()
    bias_h = nc.dram_tensor("bias_h", [QH, NCC, C], F32,
                            kind="ExternalInput").ap()
    out = nc.dram_tensor("out", [ROWS, C], F32, kind="ExternalOutput").ap()
    with tile.TileContext(nc) as tc:
        _build_body(tc, resid, wfcT, wprojT, g_bc, bias_h, out, reps, variant)
    nc.compile()
    _NC_CACHE[key] = nc
    return nc


def _in_maps(residual, w_fc, w_proj, ln_g, ln_b):
    resid2d = np.ascontiguousarray(residual.reshape(-1, C))
    wfcT = np.ascontiguousarray(w_fc.T)
    wprojT = np.ascontiguousarray(w_proj.T)
    # ln_g replicated across partitions; ln_b folded into a per-score bias:
    # score[s=(cc,r)] += sum_d ln_b[h*D+d] * w_fc[r, cc*D+d]
    g_bc = np.broadcast_to(np.asarray(ln_g)[None, :], (P, C))
    b4 = np.asarray(ln_b, np.float32).reshape(QH, D)
    wf4 = np.asarray(w_fc, np.float32).reshape(C, NCC, D)
    bias_h = np.einsum("hd,rcd->hcr", b4, wf4).astype(np.float32)
    bias_h = np.ascontiguousarray(bias_h)
    if CONFIG["bf16"]:
        import ml_dtypes
        wfcT = wfcT.astype(ml_dtypes.bfloat16)
        wprojT = wprojT.astype(ml_dtypes.bfloat16)
        g_bc = g_bc.astype(ml_dtypes.bfloat16)
    else:
        g_bc = g_bc.astype(np.float32)
    g_bc = np.ascontiguousarray(g_bc)
    return [
        {"residual": resid2d[i * ROWS:(i + 1) * ROWS],
         "w_fcT": wfcT, "w_projT": wprojT, "g_bc": g_bc, "bias_h": bias_h}
        for i in range(N_CORES)
    ]


def run_on_cores(inputs, reps=1):
    nc = build_nc(reps)
    in_maps = _in_maps(**inputs)
    return run_bass_kernel_spmd(nc, in_maps, core_ids=list(range(N_CORES)))


def kernel(residual, w_fc, w_proj, ln_g, ln_b):
    B, T, Cx = residual.shape
    res = run_on_cores(dict(residual=residual, w_fc=w_fc, w_proj=w_proj,
                            ln_g=ln_g, ln_b=ln_b))
    out = np.concatenate([r["out"] for r in res.results], axis=0)
    return out.reshape(B, T, Cx).astype(np.float32)



# revision 19
# speedup vs baseline: 1.1267x; 1.1267x over previous
"""Trainium2 Bass kernel for nn_MLPMHA (sparse_attention / squared-ReLU MLP-MHA).

Reference computation (B=4, T=2048, C=1024, QH=4, D=256, S=4C=4096):
    x   = layernorm(residual) * g + b
    q_h = x[:, h*D:(h+1)*D]                     per head h
    k   = w_fc.reshape(S, D)                    keys   (shared across heads)
    v   = w_proj.T.reshape(S, D)                values (shared across heads)
    out = residual + concat_h( relu(q_h @ k.T)^2 @ v )

Equivalent blocked form used here (cc = 0..3 indexes 256-wide column chunks
of w_fc / row chunks of w_proj; all matmuls are plain GEMMs):
    A_{h,cc}  = x_h @ w_fc[:, cc*D:(cc+1)*D].T          (T, C)
    out_h     = sum_cc relu(A_{h,cc})^2 @ w_proj[cc*D:(cc+1)*D, :].T   (T, D)

Sharding: pure data parallel over the 8192 = B*T token rows; each of the 8
cores processes 1024 rows with full (transposed) weights resident in SBUF.

On-core dataflow (PSUM accumulation fp32; matmul operands bf16 by default —
1 cycle/row on the PE like f32r, but half the SBUF/DMA traffic, FWL weight
loads, 2x DVE throughput, and 1-cycle PE transposes; measured accuracy cost
~3e-3 relative vs the 2e-2 gate):
    phase A: DMA residual rows into a persistent buffer, LayerNorm
             (bn_stats, in two tile-groups), apply ln_g via one broadcast
             multiply (ln_b is folded into a per-score bias = ln_b @ w_fc,
             host-precomputed), then one 3D DMA-xbar transpose per token
             tile moves xn into xT[c, t] layout (a_dmat; no PE cycles).
    phase B: per (h, cc, i-chunk): A^T tile = wfcT_chunk.T @ xT  (PSUM),
             relu(.+bias)^2 via ACT-relu + DVE-square (recipe mix),
             out^T PSUM accumulation over all (cc, i).
    phase C: drain out^T PSUM via ACT/DVE copies (oc_split), one 3D
             DMA-xbar transpose per (dd, tch) back to [t, c] (epi_dmat2),
             DVE-add into the residual buffer, DMA out per 4-tile group.
             The PE stream is pure matmul+ldweights (1024 + 1024 instrs);
             measured ~27 ns/PE-instr of fixed overhead puts the stream
             floor at ~273 us (mm_only variant) vs the 218 us row count.

For the benchmark reps-loop, consecutive reps are software-pipelined across
double-buffered residual/xT sets: rep i+1's phase A is emitted as closures
that ride inside rep i's phase-B instruction streams, so its DMA/LN work
hides behind rep i's matmuls ("xrep").
"""

import numpy as np

import concourse.bass as bass
import concourse.tile as tile
from concourse import mybir, bacc
from concourse.bass_utils import run_bass_kernel_spmd
from concourse.masks import make_identity

P = 128
C = 1024
D = 256
QH = 4
NCC = 4          # column chunks of w_fc (S = NCC * C kv entries)
N_CORES = 8
ROWS = 1024      # token rows per core (8192 / 8)
NT = ROWS // P   # 8 row tiles per core
EPS = 1e-5

F32 = mybir.dt.float32
F32R = mybir.dt.float32r
BF16 = mybir.dt.bfloat16

_NC_CACHE = {}

# tuning knobs (A/B tested on hardware)
CONFIG = {
    "lookahead": 2,        # software-pipeline depth for mm2 behind mm1
    "lookahead0": 2,       # mm2 lag ramp target at each head start (>=
                           # lookahead; gives the previous head's PSUM
                           # drain more slack before mm2 start=True)
    "pools": (2, 4, 0),    # psA, psO, psT bufs (psT=0 => share psA slots)
                           # wide_mm: po tiles are 2 banks, so psO bufs=2
                           # (use (2,4,0) when wide_mm=False)
    "pt_bufs": 4,          # ptpool bufs
    "work_bufs": 3,        # work pool bufs
    "defer_epi": True,     # emit head epilogue after next head's first mm1s
    "epi_at": 3,           # block index in next head where epilogue lands
    "bf16": True,          # bf16 matmul operands (weights, xT, pT) + transposes
    "mix": "XXXXZ",        # relu^2 recipe cycle: X = ACT-relu + DVE-square,
                           # Z = DVE-relu + DVE-square, Y = DVE-relu + ACT-sq
    "wide_mix": False,     # honor `mix` in the wide_relu path too (X/Z only)
    "xn_dve": True,        # layernorm-apply on DVE instead of ACT
    "aff_split": True,     # alternate phase-A copy-backs between ACT and DVE
    "interleave_a": True,  # ride phase-A och4-7 transposes in the PE stream
    "xrep": True,          # software-pipeline reps across double buffers
    "xa_stride": 8,        # blocks between cross-rep phase-A closures
    "epi_dmat": False,     # epilogue transposes via DMA xbar instead of PE
    "w_reorder": True,     # mm1 shares each stationary across token halves
    "wide_relu": True,     # one 1024-wide relu^2 per block (2-bank psA tiles)
    "wide_mm": False,      # 1024-wide matmuls: REJECTED by walrus ISA check
                           # (moving >512 => out crosses a PSUM bank); keep off
    "a_dmat": True,        # phase-A transposes via one 3D DMA-xbar transpose
                           # per token tile (frees PE + the ACT/DVE copy-backs)
    "epi_dmat2": True,     # epilogue transposes via one 3D DMA-xbar transpose
                           # per (dd, tch) on the ACT HWDGE queue
    "oc_split": False,     # epilogue PSUM-drain copies alternate ACT/DVE;
                           # measured neutral-to-worse on min-slope, keep off
}


def _phase_a_closures(env, buf, defer_grp1):
    """Phase A for buffer set `buf` as an ordered list of closures.

    Returns (closures, a_jobs): running every closure in order (at any
    spacing) then every job emits the full LayerNorm + transpose of x into
    buf['xT'].  a_jobs (och 4-7 transpose groups) may be deferred into the
    same rep's phase-B stream when `defer_grp1`.
    """
    nc, work, cfg = env["nc"], env["work"], CONFIG
    TDT = BF16 if cfg["bf16"] else F32
    resid_sb, par = buf["resid"], buf["par"]
    resid, gbc_sb, eps_t = env["resid"], env["gbc_sb"], env["eps_t"]

    stats_all = work.tile([P, NT, 2, 6], F32, name=f"sta{par}",
                          tag=f"stats{par}", bufs=1)
    mv_all = work.tile([P, NT, 2], F32, name=f"mva{par}", tag=f"mv{par}",
                       bufs=1)
    nmr_all = work.tile([P, NT], F32, name=f"nmra{par}", tag=f"nmr{par}",
                        bufs=1)
    xn_tiles = {}
    cls = []

    def _dmas():
        for tt in range(NT):
            nc.sync.dma_start(resid_sb[:, tt, :],
                              resid[tt * P:(tt + 1) * P, :])
    cls.append(_dmas)

    def _bn(tg):
        for tt in range(tg * 4, (tg + 1) * 4):
            nc.vector.bn_stats(stats_all[:, tt, 0, :], resid_sb[:, tt, 0:512])
            nc.vector.bn_stats(stats_all[:, tt, 1, :],
                               resid_sb[:, tt, 512:1024])

    def _fin(tg):
        tsl4 = slice(tg * 4, (tg + 1) * 4)
        for tt in range(tg * 4, (tg + 1) * 4):
            nc.vector.bn_aggr(mv_all[:, tt, :], stats_all[:, tt, :, :])
        # rstd = 1/sqrt(var+eps), nmr = -mu*rstd (xn = r*rstd + nmr)
        nc.scalar.activation(mv_all[:, tsl4, 1], mv_all[:, tsl4, 1],
                             mybir.ActivationFunctionType.Sqrt,
                             bias=eps_t[:], scale=1.0)
        nc.vector.reciprocal(mv_all[:, tsl4, 1], mv_all[:, tsl4, 1])
        nc.vector.tensor_tensor(out=nmr_all[:, tsl4], in0=mv_all[:, tsl4, 0],
                                in1=mv_all[:, tsl4, 1],
                                op=mybir.AluOpType.mult)
        nc.vector.tensor_scalar_mul(out=nmr_all[:, tsl4],
                                    in0=nmr_all[:, tsl4], scalar1=-1.0)

    def _xn(tt):
        xn = work.tile([P, C], TDT, name=f"xn{par}_{tt}", tag=f"xn{par}",
                       bufs=NT)
        xn_tiles[tt] = xn
        if cfg["xn_dve"]:
            nc.vector.tensor_scalar(out=xn[:], in0=resid_sb[:, tt, :],
                                    scalar1=mv_all[:, tt, 1:2],
                                    scalar2=nmr_all[:, tt:tt + 1],
                                    op0=mybir.AluOpType.mult,
                                    op1=mybir.AluOpType.add)
        else:
            nc.scalar.activation(xn[:], resid_sb[:, tt, :],
                                 mybir.ActivationFunctionType.Identity,
                                 bias=nmr_all[:, tt:tt + 1],
                                 scale=mv_all[:, tt, 1:2])
        # x_hat * g: ln_g varies along the free (channel) axis here, so one
        # broadcast-multiply handles all 8 chunks (ln_b is folded into the
        # score bias bias_sb = ln_b @ w_fc, applied at the relu)
        nc.vector.tensor_mul(out=xn[:], in0=xn[:], in1=gbc_sb[:])

    def _tgroup(tt, grp):
        if cfg["a_dmat"]:
            # one DMA-xbar transpose moves the whole [128-token, 1024-ch]
            # xn tile into xT layout: out[p, och, j] = xn[j, och*128 + p]
            # (grp carries the full tile when 0; grp 1 is a no-op)
            if grp == 0:
                nc.sync.dma_start_transpose(
                    buf["xT"][:, 0:8, tt * P:(tt + 1) * P], xn_tiles[tt][:])
            return
        # transposes grouped 4-per-PSUM-bank with one grouped copy, to cut
        # the non-PE op count (errata makes small ops expensive)
        psT, _ptag, ident = env["psT"], env["_ptag"], env["ident"]
        pst = psT.tile([P, 512], TDT, name=f"psx{par}_{tt}_{grp}",
                       tag=_ptag)
        xn = xn_tiles[tt]
        for k in range(4):
            och = grp * 4 + k
            nc.tensor.transpose(pst[:, k * P:(k + 1) * P],
                                xn[:, och * P:(och + 1) * P], ident[:])
        dst = buf["xT"][:, grp * 4:(grp + 1) * 4, tt * P:(tt + 1) * P]
        if cfg["aff_split"] and (tt + grp) % 2 == 1:
            nc.vector.tensor_copy(dst, pst.rearrange("p (k c) -> p k c", k=4))
        else:
            nc.scalar.activation(dst, pst.rearrange("p (k c) -> p k c", k=4),
                                 mybir.ActivationFunctionType.Identity)

    # stats + xn in two tile-groups so tiles 0-3 are not gated on the last
    # residual DMA; och-0-3 transposes come first (heads 0/1's channels)
    for tg in range(2):
        cls.append(lambda tg=tg: _bn(tg))
        cls.append(lambda tg=tg: _fin(tg))
        for tt in range(tg * 4, (tg + 1) * 4):
            cls.append(lambda tt=tt: _xn(tt))
    for tt in range(NT):
        cls.append(lambda tt=tt: _tgroup(tt, 0))
    a_jobs = [(tt, 1) for tt in range(NT)]
    if not defer_grp1 or cfg["a_dmat"]:
        if not cfg["a_dmat"]:
            for tt in range(NT):
                cls.append(lambda tt=tt: _tgroup(tt, 1))
        a_jobs = []
    return cls, a_jobs, _tgroup


def _phase_bc(env, buf, rep, variant='full', a_jobs=(), tgroup_fn=None,
              nbuf_closures=()):
    """Phase B (matmuls + relu^2 + out^T accumulation) + phase C for buffer
    set `buf`.  Optionally interleaves same-rep transpose jobs (`a_jobs`)
    and the next rep's phase-A closures (`nbuf_closures`) into the block
    stream."""
    nc, work, ptpool = env["nc"], env["work"], env["ptpool"]
    psA, psO, psT = env["psA"], env["psO"], env["psT"]
    wfcT_sb, wprojT_sb = env["wfcT_sb"], env["wprojT_sb"]
    bias_sb, ident, out = env["bias_sb"], env["ident"], env["out"]
    _ptag, pT_dummy = env["_ptag"], env["pT_dummy"]
    cfg = CONFIG
    MDT = BF16 if cfg["bf16"] else F32R
    TDT = BF16 if cfg["bf16"] else F32
    xT_sb, resid_sb, par = buf["xT"], buf["resid"], buf["par"]
    a_jobs = list(a_jobs)
    nbuf_closures = list(nbuf_closures)

    pending_epi = [None]

    def _flush_epi():
        if pending_epi[0] is not None:
            pending_epi[0]()
            pending_epi[0] = None

    tile_ctr = [0]
    blk_ctr = [0]

    for h in range(0 if variant == 'a_only' else QH):
        if cfg["wide_mm"]:
            # po[dd] is one 2-bank PSUM tile covering all 1024 tokens
            po = [psO.tile([P, 1024], F32, name=f"po{par}_{rep}_{h}_{dd}",
                           tag="po") for dd in range(2)]
        else:
            po = [[psO.tile([P, 512], F32,
                            name=f"po{par}_{rep}_{h}_{dd}_{tch}",
                            tag="po") for tch in range(2)] for dd in range(2)]

        def _mm1_raw(cc, ich):
            """mm1 without the nonlinearity (diagnostic variants)."""
            isl = slice(ich * P, (ich + 1) * P)
            if cfg["wide_mm"]:
                ps = psA.tile([P, 1024], F32,
                              name=f"psaq{par}_{rep}_{h}_{cc}_{ich}",
                              tag="psa")
                for wi in range(2):
                    nc.tensor.matmul(ps[:], wfcT_sb[:, cc * 2 + wi, isl],
                                     xT_sb[:, h * 2 + wi, :],
                                     start=(wi == 0), stop=(wi == 1))
                return
            for tch in range(2):
                ps = psA.tile([P, 512], F32,
                              name=f"psaq{par}_{rep}_{h}_{cc}_{ich}_{tch}",
                              tag="psa")
                tsl = slice(tch * 512, (tch + 1) * 512)
                nc.tensor.matmul(ps[:], wfcT_sb[:, cc * 2 + 0, isl],
                                 xT_sb[:, h * 2 + 0, tsl],
                                 start=True, stop=False)
                nc.tensor.matmul(ps[:], wfcT_sb[:, cc * 2 + 1, isl],
                                 xT_sb[:, h * 2 + 1, tsl],
                                 start=False, stop=True)

        def _mm1(cc, ich):
            pT = ptpool.tile([P, ROWS], MDT,
                             name=f"pT{par}_{rep}_{h}_{cc}_{ich}", tag="pT")
            isl = slice(ich * P, (ich + 1) * P)
            bias_ap = bias_sb[:, h, cc, ich:ich + 1]
            if cfg["wide_relu"]:
                # one 2-bank PSUM tile for both token halves -> a single
                # 1024-wide relu + square per block (halves ACT/DVE op
                # count and engine energy; each matmul still writes within
                # one bank)
                ps = psA.tile([P, 1024], F32,
                              name=f"psw{par}_{rep}_{h}_{cc}_{ich}",
                              tag="psa")
                if cfg["wide_mm"]:
                    # one 1024-wide matmul per contraction chunk
                    for wi in range(2):
                        nc.tensor.matmul(ps[:], wfcT_sb[:, cc * 2 + wi, isl],
                                         xT_sb[:, h * 2 + wi, :],
                                         start=(wi == 0), stop=(wi == 1))
                else:
                    for wi in range(2):
                        for tch in range(2):
                            tsl = slice(tch * 512, (tch + 1) * 512)
                            nc.tensor.matmul(ps[:, tsl],
                                             wfcT_sb[:, cc * 2 + wi, isl],
                                             xT_sb[:, h * 2 + wi, tsl],
                                             start=(wi == 0), stop=(wi == 1))
                rl = work.tile([P, 1024], MDT,
                               name=f"rlw{par}_{rep}_{h}_{cc}_{ich}",
                               tag="rl")
                tile_ctr[0] += 1
                recipe = (cfg["mix"][tile_ctr[0] % len(cfg["mix"])]
                          if cfg.get("wide_mix") else "X")
                if recipe == "Z":
                    nc.vector.tensor_scalar(out=rl[:], in0=ps[:],
                                            scalar1=bias_ap, scalar2=0.0,
                                            op0=mybir.AluOpType.add,
                                            op1=mybir.AluOpType.max)
                else:
                    nc.scalar.activation(rl[:], ps[:],
                                         mybir.ActivationFunctionType.Relu,
                                         bias=bias_ap, scale=1.0)
                nc.vector.tensor_mul(out=pT[:], in0=rl[:], in1=rl[:])
                return pT
            pss = []
            if cfg["w_reorder"]:
                # both token halves against one stationary before switching:
                # halves the LDWEIGHTS rate (interleaved accumulation groups
                # in two PSUM banks)
                for tch in range(2):
                    pss.append(psA.tile(
                        [P, 512], F32,
                        name=f"psa{par}_{rep}_{h}_{cc}_{ich}_{tch}",
                        tag="psa"))
                for wi in range(2):
                    for tch in range(2):
                        tsl = slice(tch * 512, (tch + 1) * 512)
                        nc.tensor.matmul(pss[tch][:],
                                         wfcT_sb[:, cc * 2 + wi, isl],
                                         xT_sb[:, h * 2 + wi, tsl],
                                         start=(wi == 0), stop=(wi == 1))
            for tch in range(2):
                if cfg["w_reorder"]:
                    ps = pss[tch]
                else:
                    ps = psA.tile(
                        [P, 512], F32,
                        name=f"psa{par}_{rep}_{h}_{cc}_{ich}_{tch}",
                        tag="psa")
                    tsl = slice(tch * 512, (tch + 1) * 512)
                    nc.tensor.matmul(ps[:], wfcT_sb[:, cc * 2 + 0, isl],
                                     xT_sb[:, h * 2 + 0, tsl],
                                     start=True, stop=False)
                    nc.tensor.matmul(ps[:], wfcT_sb[:, cc * 2 + 1, isl],
                                     xT_sb[:, h * 2 + 1, tsl],
                                     start=False, stop=True)
                tsl = slice(tch * 512, (tch + 1) * 512)
                # pT = relu(ps + bias)^2; recipe per tile from cfg["mix"]
                # (bias = ln_b @ w_fc chunk, the folded layernorm bias)
                tile_ctr[0] += 1
                recipe = cfg["mix"][tile_ctr[0] % len(cfg["mix"])]
                rl = work.tile([P, 512], MDT,
                               name=f"rl{par}_{rep}_{h}_{cc}_{ich}_{tch}",
                               tag="rl")
                if recipe == "X":
                    nc.scalar.activation(rl[:], ps[:],
                                         mybir.ActivationFunctionType.Relu,
                                         bias=bias_ap, scale=1.0)
                    nc.vector.tensor_mul(out=pT[:, tsl], in0=rl[:], in1=rl[:])
                elif recipe == "Z":
                    nc.vector.tensor_scalar(out=rl[:], in0=ps[:],
                                            scalar1=bias_ap, scalar2=0.0,
                                            op0=mybir.AluOpType.add,
                                            op1=mybir.AluOpType.max)
                    nc.vector.tensor_mul(out=pT[:, tsl], in0=rl[:], in1=rl[:])
                else:  # Y
                    nc.vector.tensor_scalar(out=rl[:], in0=ps[:],
                                            scalar1=bias_ap, scalar2=0.0,
                                            op0=mybir.AluOpType.add,
                                            op1=mybir.AluOpType.max)
                    nc.scalar.activation(pT[:, tsl], rl[:],
                                         mybir.ActivationFunctionType.Square)
            return pT

        def _mm2(cc, ich, pT):
            first = (cc == 0 and ich == 0)
            last = (cc == NCC - 1 and ich == 7)
            for dd in range(2):
                wsl = slice(cc * D + dd * P, cc * D + (dd + 1) * P)
                if cfg["wide_mm"]:
                    nc.tensor.matmul(po[dd][:], wprojT_sb[:, ich, wsl],
                                     pT[:], start=first, stop=last)
                    continue
                # tch inner: consecutive matmuls share the stationary
                # operand, halving the LDWEIGHTS rate
                for tch in range(2):
                    tsl = slice(tch * 512, (tch + 1) * 512)
                    nc.tensor.matmul(po[dd][tch][:],
                                     wprojT_sb[:, ich, wsl], pT[:, tsl],
                                     start=first, stop=last)

        pending = []
        nblk = 0
        for cc in range(NCC):
            for ich in range(8):
                nblk += 1
                blk_ctr[0] += 1
                if nblk == cfg["epi_at"]:
                    # previous head's epilogue lands here, hidden behind
                    # this head's first mm1 blocks
                    _flush_epi()
                if a_jobs and blk_ctr[0] % 4 == 0:
                    # a deferred phase-A transpose group rides along in the
                    # PE stream (och 4-7 are first consumed by head 2)
                    tgroup_fn(*a_jobs.pop(0))
                if nbuf_closures and blk_ctr[0] % cfg["xa_stride"] == 0:
                    # the next rep's phase-A work rides along too (it
                    # touches only the other buffer set)
                    nbuf_closures.pop(0)()
                if variant == 'mm_only':
                    # detached: mm2 reads a pre-set dummy, so PE runs the
                    # pure matmul stream with no DVE/ACT deps
                    _mm2(cc, ich, pT_dummy)
                    _mm1_raw(cc, ich)
                    continue
                pT = _mm1(cc, ich)
                if variant == 'mm1_only':
                    continue
                pending.append((cc, ich, pT))
                la = max(cfg["lookahead"], cfg.get("lookahead0", 0))
                if len(pending) > la:
                    _mm2(*pending.pop(0))
        for args in pending:
            _mm2(*args)

        # epilogue for head h: transpose out^T back, add into resid_sb;
        # after the last head's adds, each 4-token-tile group stores out
        def _epilogue(h=h, po=po):
            if variant not in ('full', 'b_only'):
                return
            for tch in range(2):
                tts = slice(tch * 4, (tch + 1) * 4)
                for dd in range(2):
                    po_src = (po[dd][:, tch * 512:(tch + 1) * 512]
                              if cfg["wide_mm"] else po[dd][tch][:])
                    oc = work.tile([P, 512], TDT,
                                   name=f"oc{par}_{rep}_{h}_{dd}_{tch}",
                                   tag="oc")
                    if cfg["oc_split"] and dd == 1:
                        nc.vector.tensor_copy(oc[:], po_src)
                    else:
                        nc.scalar.activation(
                            oc[:], po_src,
                            mybir.ActivationFunctionType.Identity)
                    csl0 = h * D + dd * P
                    csl = slice(csl0, csl0 + P)
                    if CONFIG["epi_dmat2"]:
                        # one 3D DMA-xbar transpose per (dd, tch):
                        # ocT[p, k, j] = oc[j, k*128 + p]  (contiguous dst)
                        ocT = work.tile([P, 512], TDT,
                                        name=f"ocU{par}_{rep}_{h}_{dd}_{tch}",
                                        tag="ocT")
                        nc.scalar.dma_start_transpose(
                            ocT.rearrange("p (k c) -> p k c", k=4), oc[:])
                        nc.vector.tensor_add(
                            out=resid_sb[:, tts, csl],
                            in0=resid_sb[:, tts, csl],
                            in1=ocT.rearrange("p (k c) -> p k c", k=4))
                    elif CONFIG["epi_dmat"]:
                        # transpose via the DMA xbar (idle ACT queue) to
                        # free PE cycles; SBUF->SBUF, 2-byte dtype only
                        ocT = work.tile([P, 512], TDT,
                                        name=f"ocT{par}_{rep}_{h}_{dd}_{tch}",
                                        tag="ocT")
                        for ts4 in range(4):
                            nc.scalar.dma_start_transpose(
                                ocT[:, ts4 * P:(ts4 + 1) * P],
                                oc[:, ts4 * P:(ts4 + 1) * P])
                        nc.vector.tensor_add(
                            out=resid_sb[:, tts, csl],
                            in0=resid_sb[:, tts, csl],
                            in1=ocT.rearrange("p (k c) -> p k c", k=4))
                    else:
                        pst = psT.tile([P, 512], TDT,
                                       name=f"pso{par}_{rep}_{h}_{dd}_{tch}",
                                       tag=_ptag)
                        for ts4 in range(4):
                            nc.tensor.transpose(
                                pst[:, ts4 * P:(ts4 + 1) * P],
                                oc[:, ts4 * P:(ts4 + 1) * P], ident[:])
                        # one grouped add over the 4 token tiles
                        nc.vector.tensor_add(
                            out=resid_sb[:, tts, csl],
                            in0=resid_sb[:, tts, csl],
                            in1=pst.rearrange("p (k c) -> p k c", k=4))
                if h == QH - 1:
                    for tt in range(tch * 4, (tch + 1) * 4):
                        nc.sync.dma_start(out[tt * P:(tt + 1) * P, :],
                                          resid_sb[:, tt, :])

        if cfg["defer_epi"]:
            _flush_epi()
            pending_epi[0] = _epilogue
        else:
            _epilogue()
    _flush_epi()
    for job in a_jobs:
        tgroup_fn(*job)
    for cl in nbuf_closures:
        cl()


def _phase_abc(env, buf, rep, variant='full'):
    """One full rep: inline phase A (optionally deferring och-4-7 transpose
    groups into this rep's own phase-B stream), then phase B/C."""
    skip_a = variant in ('b_only', 'mm_only', 'mm1_only')
    a_jobs, tgroup_fn = (), None
    if not skip_a:
        defer = CONFIG["interleave_a"] and variant == 'full'
        cls, a_jobs, tgroup_fn = _phase_a_closures(env, buf, defer)
        for cl in cls:
            cl()
    _phase_bc(env, buf, rep, variant, a_jobs, tgroup_fn)


def _build_body(tc, resid, wfcT, wprojT, g_bc, bias_h, out, reps,
                variant='full'):
    nc = tc.nc
    import contextlib
    cfg = CONFIG
    MDT = BF16 if cfg["bf16"] else F32R
    ctx = contextlib.ExitStack()
    with ctx:
        singles = ctx.enter_context(tc.tile_pool(name="singles", bufs=1))
        work = ctx.enter_context(tc.tile_pool(name="work", bufs=cfg["work_bufs"]))
        ptpool = ctx.enter_context(tc.tile_pool(name="ptpool", bufs=cfg["pt_bufs"]))
        psA = ctx.enter_context(tc.tile_pool(name="psA", bufs=cfg["pools"][0], space="PSUM"))
        psO = ctx.enter_context(tc.tile_pool(name="psO", bufs=cfg["pools"][1], space="PSUM"))
        if cfg["pools"][2]:
            psT = ctx.enter_context(tc.tile_pool(name="psT", bufs=cfg["pools"][2], space="PSUM"))
        else:
            psT = psA  # transposes share the psA slots (same tag => same banks)

        # ---- resident tensors -------------------------------------------
        # weights + constants ride the Activation HWDGE queue so the
        # latency-critical residual loads (SP queue) start immediately
        wfcT_sb = singles.tile([P, 8, C], MDT)
        nc.scalar.dma_start(wfcT_sb[:], wfcT.rearrange("(o p) i -> p o i", p=P))
        # ln_g replicated across partitions (host-prepared, matmul dtype)
        gbc_sb = singles.tile([P, C], MDT)
        nc.scalar.dma_start(gbc_sb[:], g_bc)
        # per-(head, cc, i-chunk) score bias = ln_b @ w_fc (host-prepared)
        bias_sb = singles.tile([P, QH, NCC, 8], F32)
        nc.scalar.dma_start(bias_sb[:],
                            bias_h.rearrange("h n (i p) -> p h n i", p=P))
        wprojT_sb = singles.tile([P, 8, C], MDT)
        nc.scalar.dma_start(wprojT_sb[:],
                            wprojT.rearrange("(o p) i -> p o i", p=P))
        ident = singles.tile([P, P], BF16 if cfg["bf16"] else F32)
        make_identity(nc, ident[:])
        eps_t = singles.tile([P, 1], F32)
        nc.vector.memset(eps_t[:], EPS)

        xrep = cfg["xrep"] and variant == 'full' and reps > 1
        bufs = []
        for par in range(2 if xrep else 1):
            bufs.append({
                "par": par,
                "xT": singles.tile([P, 8, ROWS], MDT, name=f"xT{par}"),
                "resid": singles.tile([P, NT, C], F32, name=f"resid{par}"),
            })

        pT_dummy = None
        if variant != 'full':
            # diagnostics-only variants may skip the phases that write these
            pT_dummy = singles.tile([P, ROWS], MDT)
            nc.sync.dma_start(pT_dummy[:], wfcT[0:P, :])
            nc.sync.dma_start(bufs[0]["xT"][:],
                              wfcT.rearrange("(o p) i -> p o i", p=P))
            nc.vector.memset(bufs[0]["resid"][:], 0.0)

        env = {"nc": nc, "work": work, "ptpool": ptpool, "psA": psA,
               "psO": psO, "psT": psT, "resid": resid, "out": out,
               "wfcT_sb": wfcT_sb, "wprojT_sb": wprojT_sb,
               "gbc_sb": gbc_sb, "bias_sb": bias_sb, "ident": ident,
               "eps_t": eps_t, "pT_dummy": pT_dummy,
               "_ptag": "psa" if psT is psA else "pst"}

        # ---- phases A/B/C, repeated `reps` times for benchmarking -------
        # (each rep recomputes from the DMA'd inputs and rewrites the same
        # output, so the result stays correct for any reps >= 1).  reps > 1
        # uses a hardware loop so the instruction count stays constant.
        hint = (mybir.EngineType.PE, mybir.EngineType.Activation,
                mybir.EngineType.DVE, mybir.EngineType.SP,
                mybir.EngineType.Pool)
        nofori = cfg.get("nofori", False)
        if reps == 1:
            _phase_abc(env, bufs[0], 0, variant)
        elif not xrep:
            if nofori:
                for r in range(reps):
                    _phase_abc(env, bufs[0], r, variant)
            else:
                with tc.For_i(0, reps, 1, hint_engines=hint):
                    _phase_abc(env, bufs[0], 0, variant)
        else:
            assert reps % 2 == 0, "xrep software pipeline needs even reps"
            # prologue: phase A for buffer 0 (runs once)
            cls0, _, _ = _phase_a_closures(env, bufs[0], False)
            for cl in cls0:
                cl()

            def _iter(r):
                for par in range(2):
                    cls, jobs, tg = _phase_a_closures(env, bufs[1 - par],
                                                      False)
                    _phase_bc(env, bufs[par], r, variant,
                              a_jobs=jobs, tgroup_fn=tg, nbuf_closures=cls)

            if nofori:
                for r in range(reps // 2):
                    _iter(r)
            else:
                with tc.For_i(0, reps // 2, 1, hint_engines=hint):
                    _iter(0)


def build_nc(reps=1, variant='full'):
    key = (reps, variant, str(sorted(CONFIG.items())))
    if key in _NC_CACHE:
        return _NC_CACHE[key]
    MDT = BF16 if CONFIG["bf16"] else F32R
    nc = bacc.Bacc("TRN2", target_bir_lowering=False, debug=False,
                   num_devices=N_CORES)
    resid = nc.dram_tensor("residual", [ROWS, C], F32, kind="ExternalInput").ap()
    wfcT = nc.dram_tensor("w_fcT", [C, C], MDT, kind="ExternalInput").ap()
    wprojT = nc.dram_tensor("w_projT", [C, C], MDT, kind="ExternalInput").ap()
    g_bc = nc.dram_tensor("g_bc", [P, C], MDT, kind="ExternalInput").ap()
    bias_h = nc.dram_tensor("bias_h", [QH, NCC, C], F32,
                            kind="ExternalInput").ap()
    out = nc.dram_tensor("out", [ROWS, C], F32, kind="ExternalOutput").ap()
    with tile.TileContext(nc) as tc:
        _build_body(tc, resid, wfcT, wprojT, g_bc, bias_h, out, reps, variant)
    nc.compile()
    _NC_CACHE[key] = nc
    return nc


def _in_maps(residual, w_fc, w_proj, ln_g, ln_b):
    resid2d = np.ascontiguousarray(residual.reshape(-1, C))
    wfcT = np.ascontiguousarray(w_fc.T)
    wprojT = np.ascontiguousarray(w_proj.T)
    # ln_g replicated across partitions; ln_b folded into a per-score bias:
    # score[s=(cc,r)] += sum_d ln_b[h*D+d] * w_fc[r, cc*D+d]
    g_bc = np.broadcast_to(np.asarray(ln_g)[None, :], (P, C))
    b4 = np.asarray(ln_b, np.float32).reshape(QH, D)
    wf4 = np.asarray(w_fc, np.float32).reshape(C, NCC, D)
    bias_h = np.einsum("hd,rcd->hcr", b4, wf4).astype(np.float32)
    bias_h = np.ascontiguousarray(bias_h)
    if CONFIG["bf16"]:
        import ml_dtypes
        wfcT = wfcT.astype(ml_dtypes.bfloat16)
        wprojT = wprojT.astype(ml_dtypes.bfloat16)
        g_bc = g_bc.astype(ml_dtypes.bfloat16)
    else:
        g_bc = g_bc.astype(np.float32)
    g_bc = np.ascontiguousarray(g_bc)
    return [
        {"residual": resid2d[i * ROWS:(i + 1) * ROWS],
         "w_fcT": wfcT, "w_projT": wprojT, "g_bc": g_bc, "bias_h": bias_h}
        for i in range(N_CORES)
    ]


def run_on_cores(inputs, reps=1):
    nc = build_nc(reps)
    in_maps = _in_maps(**inputs)
    return run_bass_kernel_spmd(nc, in_maps, core_ids=list(range(N_CORES)))


def kernel(residual, w_fc, w_proj, ln_g, ln_b):
    B, T, Cx = residual.shape
    res = run_on_cores(dict(residual=residual, w_fc=w_fc, w_proj=w_proj,
                            ln_g=ln_g, ln_b=ln_b))
    out = np.concatenate([r["out"] for r in res.results], axis=0)
    return out.reshape(B, T, Cx).astype(np.float32)

